# revision 1
# baseline (speedup 1.0000x reference)
"""GPT2 eager causal attention (B=2, S=2048, D=1024, H=16, HD=64) on 8 TRN2 NeuronCores.

Sharding (data + head/tensor parallel, per the problem's hint):
  core c -> (batch b = c//4, head-group g = c%4) -- 4 heads per group.

Per-core pipeline (all layouts chosen so no score-matrix transpose is ever needed):
  1. x[b] transposed on PE -> xT [d, s]                      (d on partitions)
  2. QT,KT = wq/wk^T @ xT  -> [256, S] transposed layouts    (head-dim on partitions)
     V     = xT^T @ wv_ext -> [S, 260] natural, with a ones-column per head
  3. scores^T tiles ST[k, q] = KT_h^T-slices @ QT_h-slices   (k on partitions)
     exp on ScalarE with the 1/sqrt(64) scale folded in; causal masking via
     precomputed mask tiles on diagonal blocks only
     OT[d, q] += V^T-slices @ ST_exp : the ones-column makes row 64 the softmax
     denominator for free; normalize OT by its reciprocal (gpsimd broadcast)
  4. c_proj partial = OT^T-slices @ w_proj[group rows]
  5. ReduceScatter(add) over each 4-core (same-batch) group; each core emits its
     [512, 1024] token slice; host reassembles the [2, 2048, 1024] output.

Matmuls run as float32r (full-rate on PE for free dim >= 256, ~tf32 precision),
fp32 accumulation in PSUM, all storage fp32.
"""
from contextlib import ExitStack

import ml_dtypes
import numpy as np

import concourse.bacc as bacc
import concourse.mybir as mybir
import concourse.tile as tile
from concourse.bass_utils import run_bass_kernel_spmd

F32 = mybir.dt.float32
F32R = mybir.dt.float32r
BF16 = mybir.dt.bfloat16

B, S, D, H, HD = 2, 2048, 1024, 16, 64
N_CORES = 8
HG = 4               # heads per group
DG = HG * HD         # 256 q/k channels per group
VW = HG * (HD + 1)   # 260: 64 v-cols + 1 ones-col per head
NK = D // 128        # 8 contraction tiles over d
NS = S // 128        # 16 token tiles
CH = 512             # q-chunk (one PSUM bank of fp32)
NCH = S // CH        # 4
NRT = DG // 128      # 2 channel row-tiles per group


def _build(has_bv: bool, has_bp: bool, has_bqk: bool = False, tail: str = "rs", phases: int = 99):
    nc = bacc.Bacc("TRN2", target_bir_lowering=False, debug=False, num_devices=N_CORES)

    x_d = nc.dram_tensor("x", [S, D], BF16, kind="ExternalInput").ap()
    wq_d = nc.dram_tensor("wq", [D, DG], BF16, kind="ExternalInput").ap()
    wk_d = nc.dram_tensor("wk", [D, DG], BF16, kind="ExternalInput").ap()
    wv_d = nc.dram_tensor("wv", [D, VW], BF16, kind="ExternalInput").ap()
    wp_d = nc.dram_tensor("wp", [DG, D], BF16, kind="ExternalInput").ap()
    bq_d = nc.dram_tensor("bq", [DG, 1], F32, kind="ExternalInput").ap()
    bk_d = nc.dram_tensor("bk", [DG, 1], F32, kind="ExternalInput").ap()
    bv_d = nc.dram_tensor("bv", [DG, 1], F32, kind="ExternalInput").ap()
    bp_d = nc.dram_tensor("bp", [128, D], F32, kind="ExternalInput").ap()
    mk_d = nc.dram_tensor("masks", [128, 128], BF16, kind="ExternalInput").ap()
    if tail == "rs":
        out_d = nc.dram_tensor("out", [CH, D], F32, kind="ExternalOutput").ap()
    else:  # debug: emit the full per-core partial
        out_d = nc.dram_tensor("out", [S, D], F32, kind="ExternalOutput").ap()

    EXP = mybir.ActivationFunctionType.Exp
    IDENT = mybir.ActivationFunctionType.Identity

    with ExitStack() as ctx:
        tc = ctx.enter_context(tile.TileContext(nc))
        wpool = ctx.enter_context(tc.tile_pool(name="w", bufs=1))
        big = ctx.enter_context(tc.tile_pool(name="big", bufs=8))
        qkvp = ctx.enter_context(tc.tile_pool(name="qkv", bufs=1))
        stp = ctx.enter_context(tc.tile_pool(name="stx", bufs=6))
        nrm = ctx.enter_context(tc.tile_pool(name="nrm", bufs=2))
        outp = ctx.enter_context(tc.tile_pool(name="outp", bufs=3))
        ps_mm = ctx.enter_context(tc.tile_pool(name="psmm", bufs=3, space="PSUM"))
        ps_st = ctx.enter_context(tc.tile_pool(name="psst", bufs=3, space="PSUM"))
        ps_ot = ctx.enter_context(tc.tile_pool(name="psot", bufs=2, space="PSUM"))
        dram = ctx.enter_context(tc.tile_pool(name="dram", bufs=1, space="DRAM"))

        # ---- constants / weights -> SBUF
        wq_sb = wpool.tile([128, NK * DG], BF16)
        wk_sb = wpool.tile([128, NK * DG], BF16)
        wv_sb = wpool.tile([128, NK * VW], BF16)
        wp_sb = wpool.tile([128, NRT * D], BF16)
        mk_sb = wpool.tile([128, 128], BF16)
        on_sb = wpool.tile([1, 64], F32)
        bq_sb = wpool.tile([128, NRT], F32)
        bk_sb = wpool.tile([128, NRT], F32)
        bv_sb = wpool.tile([128, NRT], F32) if has_bv else None
        bp_sb = wpool.tile([128, D], F32) if has_bp else None
        for kt in range(NK):
            nc.sync.dma_start(wq_sb[:, kt * DG:(kt + 1) * DG], wq_d[kt * 128:(kt + 1) * 128, :])
            nc.sync.dma_start(wk_sb[:, kt * DG:(kt + 1) * DG], wk_d[kt * 128:(kt + 1) * 128, :])
            nc.sync.dma_start(wv_sb[:, kt * VW:(kt + 1) * VW], wv_d[kt * 128:(kt + 1) * 128, :])
        for rt in range(NRT):
            nc.sync.dma_start(wp_sb[:, rt * D:(rt + 1) * D], wp_d[rt * 128:(rt + 1) * 128, :])
            nc.sync.dma_start(bq_sb[:, rt:rt + 1], bq_d[rt * 128:(rt + 1) * 128, :])
            nc.sync.dma_start(bk_sb[:, rt:rt + 1], bk_d[rt * 128:(rt + 1) * 128, :])
            if has_bv:
                nc.sync.dma_start(bv_sb[:, rt:rt + 1], bv_d[rt * 128:(rt + 1) * 128, :])
        if has_bp:
            nc.sync.dma_start(bp_sb[:], bp_d[:])
        nc.sync.dma_start(mk_sb[:], mk_d[:])
        nc.vector.memset(on_sb[:], 1.0)

        # ---- phase 1: xT strips [128 d, S] via transpose-DMA (bf16 xbar path),
        # split into column chunks; weights were enqueued first so QKV can
        # start as soon as the sq=0 chunks land
        xT = []
        for dt in range(NK):
            t = big.tile([128, S], BF16, tag="bigslot", name=f"xT{dt}")
            xT.append(t)
        for sq in range(4):
            for dt in range(NK):
                nc.sync.dma_start_transpose(
                    xT[dt][:, sq * CH:(sq + 1) * CH],
                    x_d[sq * CH:(sq + 1) * CH, dt * 128:(dt + 1) * 128],
                )


        # ---- phase 2: QT/KT [256, S] (as 2 tiles of [128, S]) and V strips
        QT, KT = [], []
        for store, w_sb, b_sb, nm in ((QT, wq_sb, bq_sb, "q"), (KT, wk_sb, bk_sb, "k")):
            for rt in range(NRT):
                dst = qkvp.tile([128, S], BF16, tag=f"{nm}t{rt}", name=f"{nm}T{rt}")
                store.append(dst)
                for ch in range(NCH):
                    ps = ps_mm.tile([128, CH], F32, tag="ps", name=f"ps{nm}{rt}_{ch}")
                    for kt in range(NK):
                        nc.tensor.matmul(
                            ps[:],
                            (w_sb[:, kt * DG + rt * 128: kt * DG + (rt + 1) * 128]),
                            (xT[kt][:, ch * CH:(ch + 1) * CH]),
                            start=(kt == 0), stop=(kt == NK - 1),
                        )
                    if has_bqk:
                        nc.scalar.activation(
                            dst[:, ch * CH:(ch + 1) * CH], ps[:], IDENT,
                            bias=b_sb[:, rt:rt + 1],
                        )
                    else:
                        nc.vector.tensor_copy(dst[:, ch * CH:(ch + 1) * CH], ps[:])
        V = []
        for st in range(NS):
            vt = qkvp.tile([128, VW], BF16, tag=f"v{st}", name=f"v{st}")
            ps = ps_mm.tile([128, CH], F32, tag="ps", name=f"psv{st}")
            for kt in range(NK):
                nc.tensor.matmul(
                    ps[:, :VW],
                    (xT[kt][:, st * 128:(st + 1) * 128]),
                    (wv_sb[:, kt * VW:(kt + 1) * VW]),
                    start=(kt == 0), stop=(kt == NK - 1),
                )
            nc.vector.tensor_copy(vt[:], ps[:, :VW])
            for hl in range(HG):
                ones_col = vt[:, hl * (HD + 1) + HD: (hl + 1) * (HD + 1)].bitcast(mybir.dt.uint16)
                nc.vector.memset(ones_col, 0x3F80)  # bits of bf16 1.0
            V.append(vt)

        # ---- phase 3: attention + c_proj, chunk by chunk
        OT = []
        for i in range(NRT):
            t = big.tile([128, S], BF16, tag="bigslot", name=f"OT{i}")
            OT.append(t)
        partials = []
        for ch in range(NCH):
            pt = dram.tile([CH, D], F32, tag=f"partial{ch}", name=f"partial{ch}")
            partials.append(pt)
        rs_outs = []

        def emit_rs(ch):
            # rank r of the quad receives tokens [512*ch + 128*r, +128)
            rs_c = dram.tile([128, D], F32, tag=f"rs{ch}", name=f"rs_out{ch}")
            nc.gpsimd.collective_compute(
                "ReduceScatter",
                mybir.AluOpType.add,
                replica_groups=[[0, 1, 2, 3], [4, 5, 6, 7]],
                ins=[partials[ch].opt()],
                outs=[rs_c.opt()],
            )
            rs_outs.append((ch, rs_c))
        for ch in range(NCH):
            nkt = 4 * (ch + 1)
            for hl in range(HG):
                qt = QT[hl // 2]
                ktile = KT[hl // 2]
                off = 64 * (hl % 2)
                ot_ps = ps_ot.tile([65, CH], F32, tag="ot", name=f"ot{ch}_{hl}")
                for kt in range(nkt):
                    st_ps = ps_st.tile([128, CH], F32, tag="st", name=f"st{ch}_{hl}_{kt}")
                    nc.tensor.matmul(
                        st_ps[:],
                        (ktile[off:off + 64, kt * 128:(kt + 1) * 128]),
                        (qt[off:off + 64, ch * CH:(ch + 1) * CH]),
                        start=True, stop=True,
                    )
                    st_sb = stp.tile([128, CH], BF16, tag="stsb", name=f"se{ch}_{hl}_{kt}")
                    d = kt - 4 * ch
                    if d < 0:
                        nc.scalar.activation(st_sb[:], st_ps[:], EXP, scale=0.125)
                    else:
                        # diagonal strip: exp only the valid suffix, zero the
                        # prefix, triangular-mask the 128-wide diagonal block
                        if d > 0:
                            zc = st_sb[:, 0:d * 128].bitcast(mybir.dt.uint16)
                            nc.vector.memset(zc, 0)
                        nc.scalar.activation(st_sb[:, d * 128:], st_ps[:, d * 128:], EXP, scale=0.125)
                        nc.vector.tensor_mul(
                            st_sb[:, d * 128:(d + 1) * 128],
                            st_sb[:, d * 128:(d + 1) * 128],
                            mk_sb[:, 0:128],
                        )
                    nc.tensor.matmul(
                        ot_ps[:],
                        (V[kt][:, hl * (HD + 1):(hl + 1) * (HD + 1)]),
                        (st_sb[:]),
                        start=(kt == 0), stop=(kt == nkt - 1),
                    )
                den = nrm.tile([1, CH], F32, tag="den", name=f"den{ch}_{hl}")
                nc.vector.tensor_copy(den[:], ot_ps[64:65, :])
                rden = nrm.tile([1, CH], F32, tag="rden", name=f"rden{ch}_{hl}")
                nc.vector.reciprocal_approx_fast(rden[:], den[:])
                # rank-1 PE matmul broadcasts the reciprocal row to 64
                # partitions (keeps gpsimd free for the collectives)
                rbc_ps = ps_mm.tile([64, CH], F32, tag="ps", name=f"rbc{ch}_{hl}")
                nc.tensor.matmul(rbc_ps[:], on_sb[:], rden[:], start=True, stop=True)
                ot_sb = nrm.tile([64, CH], BF16, tag="otsb", name=f"otsb{ch}_{hl}")
                nc.vector.tensor_copy(ot_sb[:], ot_ps[0:64, :])
                dst = OT[hl // 2][off:off + 64, ch * CH:(ch + 1) * CH]
                nc.vector.tensor_mul(dst, ot_sb[:], rbc_ps[:])
                if has_bv:
                    nc.vector.tensor_scalar_add(dst, dst, bv_sb[off:off + 64, hl // 2: hl // 2 + 1])
            # c_proj for this chunk's tokens
            for stl in range(4):
                tok = ch * CH + stl * 128
                for n in range(NRT):
                    po = ps_mm.tile([128, CH], F32, tag="ps", name=f"po{ch}_{stl}_{n}")
                    for k2 in range(NRT):
                        nc.tensor.matmul(
                            po[:],
                            (OT[k2][:, tok:tok + 128]),
                            (wp_sb[:, k2 * D + n * CH: k2 * D + (n + 1) * CH]),
                            start=(k2 == 0), stop=(k2 == NRT - 1),
                        )
                    ob = outp.tile([128, CH], F32, tag="ob", name=f"ob{ch}_{stl}_{n}")
                    if has_bp:
                        nc.vector.tensor_add(ob[:], po[:], bp_sb[:, n * CH:(n + 1) * CH])
                    else:
                        nc.vector.tensor_copy(ob[:], po[:])
                    if tail == "rs":
                        nc.sync.dma_start(partials[ch][stl * 128:(stl + 1) * 128, n * CH:(n + 1) * CH], ob[:])
                    else:
                        nc.sync.dma_start(out_d[tok:tok + 128, n * CH:(n + 1) * CH], ob[:])
            if tail == "rs":
                emit_rs(ch)
        if tail == "rs":
            # final out DMAs last: keeps the in-order sync queue from blocking
            # mid-kernel partial writes behind collective completion waits
            for ch, rs_c in rs_outs:
                nc.sync.dma_start(out_d[ch * 128:(ch + 1) * 128, :], rs_c[:])

    nc.compile()
    return nc


_prog_cache = {}


def _get_prog(has_bv, has_bp, has_bqk):
    key = (has_bv, has_bp, has_bqk)
    if key not in _prog_cache:
        _prog_cache[key] = _build(has_bv, has_bp, has_bqk)
    return _prog_cache[key]


def _prepare(x, w_attn, b_attn, w_proj, b_proj):
    x = np.asarray(x, dtype=np.float32)
    w_attn = np.asarray(w_attn, dtype=np.float32)
    b_attn = np.asarray(b_attn, dtype=np.float32)
    w_proj = np.asarray(w_proj, dtype=np.float32)
    b_proj = np.asarray(b_proj, dtype=np.float32)

    has_bv = bool(np.any(b_attn[2 * D:]))
    has_bp = bool(np.any(b_proj))
    has_bqk = bool(np.any(b_attn[:2 * D]))
    nc = _get_prog(has_bv, has_bp, has_bqk)

    ii = np.arange(128)[:, None]
    jj = np.arange(128)[None, :]
    masks = (jj >= ii).astype(np.float32).astype(ml_dtypes.bfloat16)

    in_maps = []
    for c in range(N_CORES):
        b, g = divmod(c, 4)
        q0 = g * DG
        k0 = D + g * DG
        v0 = 2 * D + g * DG
        wv_ext = np.zeros((D, VW), dtype=np.float32)
        for hl in range(HG):
            wv_ext[:, hl * (HD + 1):hl * (HD + 1) + HD] = w_attn[:, v0 + hl * HD: v0 + (hl + 1) * HD]
        if g == 0:
            bp_tile = np.broadcast_to(b_proj, (128, D)).astype(np.float32)
        else:
            bp_tile = np.zeros((128, D), dtype=np.float32)
        in_maps.append({
            "x": np.ascontiguousarray(x[b]).astype(ml_dtypes.bfloat16),
            "wq": np.ascontiguousarray(w_attn[:, q0:q0 + DG]).astype(ml_dtypes.bfloat16),
            "wk": np.ascontiguousarray(w_attn[:, k0:k0 + DG]).astype(ml_dtypes.bfloat16),
            "wv": wv_ext.astype(ml_dtypes.bfloat16),
            "wp": np.ascontiguousarray(w_proj[g * DG:(g + 1) * DG, :]).astype(ml_dtypes.bfloat16),
            "bq": np.ascontiguousarray(b_attn[q0:q0 + DG, None]),
            "bk": np.ascontiguousarray(b_attn[k0:k0 + DG, None]),
            "bv": np.ascontiguousarray(b_attn[v0:v0 + DG, None]),
            "bp": bp_tile,
            "masks": masks,
        })
    return nc, in_maps


def _assemble(results):
    out = np.empty((B, S, D), dtype=np.float32)
    for c in range(N_CORES):
        b, g = divmod(c, 4)
        o = results[c]["out"]
        for ch in range(NCH):
            tok = ch * CH + g * 128
            out[b, tok:tok + 128, :] = o[ch * 128:(ch + 1) * 128, :]
    return out


def kernel(x, w_attn, b_attn, w_proj, b_proj):
    nc, in_maps = _prepare(x, w_attn, b_attn, w_proj, b_proj)
    res = run_bass_kernel_spmd(nc, in_maps, list(range(N_CORES)))
    return _assemble(res.results)



# revision 21
# speedup vs baseline: 1.4687x; 1.4687x over previous
"""GPT2 eager causal attention (B=2, S=2048, D=1024, H=16, HD=64) on 8 TRN2 NeuronCores.

Sharding (data + head/tensor parallel): core c -> (batch b = c//4, head-group
g = c%4) -- 4 heads per group; each quad (same batch) exchanges attention
outputs with a small bf16 AllToAll and computes the full c_proj locally for
its own token slices (no ReduceScatter, no fp32 partials).

Per-core pipeline:
  0. host pre-transposes x -> xT[d, s] and pre-interleaves every weight into
     its SBUF layout, so all device DMAs are large and contiguous.
  1. per 512-token chunk ch: QT/KT row-tiles and V strips via PE matmuls
     (d-contraction over 8 k-tiles), interleaved with...
  2. attention for chunk ch, head-pair t: score tiles for both heads of the
     pair go into one [128,1024] 2-bank PSUM tile via two row-group-packed
     K=64 matmuls (rows 0:64 / 64:128 -> concurrent on the PE sub-arrays);
     ONE exp covers both heads; diagonal k-tiles exp only the causal suffix
     and the st/ot matmuls skip the masked prefix columns entirely.
     V carries a ones-column per head so ot row 64 is the softmax denominator;
     reciprocal straight from PSUM, broadcast to 128 partitions with a K=2
     sel-matmul, normalize on DVE.
  3. OT chunk [256, 512]bf16 -> DRAM -> AllToAll over the quad (each core
     receives the full-model OT columns for its 128-token slice) -> local
     c_proj with the full w_proj -> fp32 out rows.
"""
from contextlib import ExitStack

import ml_dtypes
import numpy as np

import concourse.bacc as bacc
import concourse.mybir as mybir
import concourse.tile as tile
from concourse.bass import AP
from concourse.bass_utils import run_bass_kernel_spmd

F32 = mybir.dt.float32
BF16 = mybir.dt.bfloat16

B, S, D, H, HD = 2, 2048, 1024, 16, 64
N_CORES = 8
HG = 4               # heads per group (per core)
DG = HG * HD         # 256 q/k channels per group
VW = HG * (HD + 1)   # 260: 64 v-cols + 1 ones-col per head
NK = D // 128        # 8 contraction tiles over d
CH = 512             # q-chunk (one PSUM bank of fp32)
NCH = S // CH        # 4
NRT = DG // 128      # 2 channel row-tiles per group
WARMUP_MM = 26       # dummy matmuls to lift the PE HAM throttle before real work


def _build(has_bv: bool, has_bp: bool, has_bqk: bool = False, dump_ot: bool = False):
    nc = bacc.Bacc("TRN2", target_bir_lowering=False, debug=False, num_devices=N_CORES)
    dbg_d = nc.dram_tensor("dbg", [128, NRT * S], F32, kind="ExternalOutput").ap() if dump_ot else None

    xt_d = nc.dram_tensor("xt", [128, NCH * NK * CH], BF16, kind="ExternalInput").ap()
    wq_d = nc.dram_tensor("wq", [128, NK * DG], BF16, kind="ExternalInput").ap()
    wk_d = nc.dram_tensor("wk", [128, NK * DG], BF16, kind="ExternalInput").ap()
    wv_d = nc.dram_tensor("wv", [128, NK * VW], BF16, kind="ExternalInput").ap()
    wp_d = nc.dram_tensor("wp", [128, NK * D], BF16, kind="ExternalInput").ap()
    bq_d = nc.dram_tensor("bq", [128, NRT], F32, kind="ExternalInput").ap()
    bk_d = nc.dram_tensor("bk", [128, NRT], F32, kind="ExternalInput").ap()
    bv_d = nc.dram_tensor("bv", [128, NRT], F32, kind="ExternalInput").ap()
    bp_d = nc.dram_tensor("bp", [128, D], F32, kind="ExternalInput").ap()
    mk_d = nc.dram_tensor("masks", [128, 256], BF16, kind="ExternalInput").ap()
    out_d = nc.dram_tensor("out", [CH, D], F32, kind="ExternalOutput").ap()

    EXP = mybir.ActivationFunctionType.Exp
    IDENT = mybir.ActivationFunctionType.Identity

    with ExitStack() as ctx:
        tc = ctx.enter_context(tile.TileContext(nc))
        wpool = ctx.enter_context(tc.tile_pool(name="w", bufs=1))
        big = ctx.enter_context(tc.tile_pool(name="big", bufs=1))
        qkvp = ctx.enter_context(tc.tile_pool(name="qkv", bufs=1))
        stp = ctx.enter_context(tc.tile_pool(name="stx", bufs=3))
        nrm = ctx.enter_context(tc.tile_pool(name="nrm", bufs=2))
        cpj = ctx.enter_context(tc.tile_pool(name="cpj", bufs=2))
        outp = ctx.enter_context(tc.tile_pool(name="outp", bufs=3))
        ps_mm = ctx.enter_context(tc.tile_pool(name="psmm", bufs=2, space="PSUM"))
        ps_st = ctx.enter_context(tc.tile_pool(name="psst", bufs=2, space="PSUM"))
        ps_ot = ctx.enter_context(tc.tile_pool(name="psot", bufs=1, space="PSUM"))
        dram = ctx.enter_context(tc.tile_pool(name="dram", bufs=1, space="DRAM"))

        # ---- tiny constants first so the PE warm-up can start immediately
        # mk_sb cols 0:128 = triangular causal mask; rows 0:2 cols 128:256 =
        # the sel pattern that broadcasts rden row 0 -> partitions 0:64 and
        # row 1 -> partitions 64:128 via a K=2 matmul.
        mk_sb = wpool.tile([128, 256], BF16)
        nc.sync.dma_start(mk_sb[:], mk_d[:])
        if WARMUP_MM:
            warm_ps = ps_mm.tile([128, 512], F32, tag="ps", name="warm_ps")
            for i in range(WARMUP_MM):
                nc.tensor.matmul(
                    warm_ps[:, 0:128], mk_sb[:, 0:128], mk_sb[:, 0:128],
                    start=True, stop=True,
                )

        # ---- weights / x strips (host already laid out in SBUF order)
        xt_sb = big.tile([128, NCH * NK * CH], BF16, name="xt_sb")
        xt3 = xt_sb.rearrange("p (k c) -> p k c", k=NK)
        wq_sb = wpool.tile([128, NK * DG], BF16)
        wk_sb = wpool.tile([128, NK * DG], BF16)
        wv_sb = wpool.tile([128, NK * VW], BF16)
        wp_sb = wpool.tile([128, NK * D], BF16)
        bq_sb = wpool.tile([128, NRT], F32) if has_bqk else None
        bk_sb = wpool.tile([128, NRT], F32) if has_bqk else None
        bv_sb = wpool.tile([128, NRT], F32) if has_bv else None
        bp_sb = wpool.tile([128, D], F32) if has_bp else None

        def load_x_chunk(ch):
            nc.sync.dma_start(
                xt3[:, :, ch * CH:(ch + 1) * CH],
                xt_d[:, ch * NK * CH:(ch + 1) * NK * CH],
            )

        load_x_chunk(0)
        nc.sync.dma_start(wq_sb[:], wq_d[:])
        nc.sync.dma_start(wk_sb[:], wk_d[:])
        nc.sync.dma_start(wv_sb[:], wv_d[:])
        for ch in range(1, NCH):
            load_x_chunk(ch)
        nc.sync.dma_start(wp_sb[:, :NK * D // 2], wp_d[:, :NK * D // 2])
        nc.sync.dma_start(wp_sb[:, NK * D // 2:], wp_d[:, NK * D // 2:])
        if has_bqk:
            nc.sync.dma_start(bq_sb[:], bq_d[:])
            nc.sync.dma_start(bk_sb[:], bk_d[:])
        if has_bv:
            nc.sync.dma_start(bv_sb[:], bv_d[:])
        if has_bp:
            nc.sync.dma_start(bp_sb[:], bp_d[:])

        # ---- persistent SBUF tensors
        QT = [qkvp.tile([128, S], BF16, name=f"qT{rt}") for rt in range(NRT)]
        KT = [qkvp.tile([128, S], BF16, name=f"kT{rt}") for rt in range(NRT)]
        V = [qkvp.tile([128, VW], BF16, tag=f"v{st}", name=f"v{st}") for st in range(S // 128)]
        OT = [big.tile([128, S], BF16, name=f"OT{t}") for t in range(NRT)]

        def qkv_chunk(ch):
            # QT/KT row-tiles for this chunk's 512 tokens
            for store, w_sb, b_sb, nm in ((KT, wk_sb, bk_sb, "k"), (QT, wq_sb, bq_sb, "q")):
                for rt in range(NRT):
                    ps = ps_mm.tile([128, CH], F32, tag="ps", name=f"ps{nm}{rt}_{ch}")
                    for kt in range(NK):
                        nc.tensor.matmul(
                            ps[:],
                            w_sb[:, kt * DG + rt * 128: kt * DG + (rt + 1) * 128],
                            xt3[:, kt, ch * CH:(ch + 1) * CH],
                            start=(kt == 0), stop=(kt == NK - 1),
                        )
                    dst = store[rt][:, ch * CH:(ch + 1) * CH]
                    if has_bqk:
                        nc.scalar.activation(dst, ps[:], IDENT, bias=b_sb[:, rt:rt + 1])
                    else:
                        nc.vector.tensor_copy(dst, ps[:])
            # V strips
            for st in range(4 * ch, 4 * ch + 4):
                ps = ps_mm.tile([128, CH], F32, tag="ps", name=f"psv{st}")
                for kt in range(NK):
                    nc.tensor.matmul(
                        ps[:, :VW],
                        xt3[:, kt, st * 128:(st + 1) * 128],
                        wv_sb[:, kt * VW:(kt + 1) * VW],
                        start=(kt == 0), stop=(kt == NK - 1),
                    )
                vt = V[st]
                nc.vector.tensor_copy(vt[:], ps[:, :VW])
                for hl in range(HG):
                    ones_col = vt[:, hl * (HD + 1) + HD: (hl + 1) * (HD + 1)].bitcast(mybir.dt.uint16)
                    nc.vector.memset(ones_col, 0x3F80)  # bf16 1.0

        # DRAM staging for the per-chunk AllGather: each core contributes its
        # [256, 512] bf16 OT chunk and receives the full-model [1024, 512];
        # it then loads only the 128-token column slice it owns (dynamic
        # offset from the device id).
        ag_ins, ag_outs = [], []
        for ch in range(NCH):
            ai = dram.tile([DG, CH], BF16, tag=f"agi{ch}", name=f"ag_in{ch}")
            ao = dram.tile([HG * DG, CH], BF16, tag=f"ago{ch}", name=f"ag_out{ch}")
            ag_ins.append(ai)
            ag_outs.append(ao)
        pid = nc.sync.partition_id()
        goff = (pid % HG) * 128  # my token-block column offset in the gathered chunk

        def attention_chunk(ch):
            q0 = ch * CH
            nkt = 4 * (ch + 1)
            for t in range(NRT):  # head pair t: heads (2t, 2t+1) of the group
                ot_ps = {}
                ot_ps[0] = ps_ot.tile([HD + 1, CH], F32, tag="otA", name=f"otA{ch}_{t}")
                ot_ps[1] = ps_ot.tile([HD + 1, CH], F32, tag="otB", name=f"otB{ch}_{t}")
                for kt in range(nkt):
                    d = kt - 4 * ch
                    d0 = max(d, 0) * 128
                    st_ps = ps_st.tile([128, 2 * CH], F32, tag="st", name=f"st{ch}_{t}_{kt}")
                    st_sb = stp.tile([128, 2 * CH], BF16, tag="stsb", name=f"se{ch}_{t}_{kt}")
                    for hh in range(2):  # row-group packed pair: concurrent on PE
                        off = 64 * hh
                        nc.tensor.matmul(
                            st_ps[:, hh * CH + d0: (hh + 1) * CH],
                            KT[t][off:off + 64, kt * 128:(kt + 1) * 128],
                            QT[t][off:off + 64, q0 + d0: q0 + CH],
                            start=True, stop=True,
                        )
                    if d <= 0:
                        # one exp covers both heads' 512-column halves
                        nc.scalar.activation(st_sb[:], st_ps[:], EXP, scale=0.125)
                    else:
                        for hh in range(2):
                            nc.scalar.activation(
                                st_sb[:, hh * CH + d0: (hh + 1) * CH],
                                st_ps[:, hh * CH + d0: (hh + 1) * CH],
                                EXP, scale=0.125,
                            )
                    if d >= 0:
                        for hh in range(2):
                            blk = st_sb[:, hh * CH + d0: hh * CH + d0 + 128]
                            nc.vector.tensor_mul(blk, blk, mk_sb[:, 0:128])
                    for hh in range(2):
                        hl = 2 * t + hh
                        nc.tensor.matmul(
                            ot_ps[hh][:, d0:],
                            V[kt][:, hl * (HD + 1):(hl + 1) * (HD + 1)],
                            st_sb[:, hh * CH + d0:(hh + 1) * CH],
                            start=(kt == 0), stop=(kt == nkt - 1),
                        )
                # normalize the pair: row 64 of each ot_ps is the denominator.
                # Engine APs must start at 32-aligned partitions, so the two
                # reciprocal rows live at partitions 0 and 32, and two K=1
                # ones-row matmuls broadcast them to partitions 0:64 / 64:128.
                # reciprocal_approx_fast silently ignores a PSUM partition
                # offset, so the denominator rows bounce through SBUF; all
                # broadcast ops stay at partition base 0 (only 0/64 shifts
                # are safe on the engines).
                den = nrm.tile([1, 2 * CH], F32, tag="den", name=f"den{ch}_{t}")
                rden = nrm.tile([1, 2 * CH], F32, tag="rden", name=f"rden{ch}_{t}")
                rden_bf = nrm.tile([1, 2 * CH], BF16, tag="rdenb", name=f"rdenb{ch}_{t}")
                nc.vector.tensor_copy(den[0:1, 0:CH], ot_ps[0][64:65, :])
                nc.vector.tensor_copy(den[0:1, CH:2 * CH], ot_ps[1][64:65, :])
                nc.vector.reciprocal_approx_fast(rden[:], den[:])
                nc.vector.tensor_copy(rden_bf[:], rden[:])
                for hh in range(2):
                    rbc_ps = ps_mm.tile([64, CH], F32, tag="ps", name=f"rbc{ch}_{t}_{hh}")
                    nc.tensor.matmul(
                        rbc_ps[:],
                        mk_sb[0:1, 128:192],
                        rden_bf[0:1, hh * CH:(hh + 1) * CH],
                        start=True, stop=True,
                    )
                    rbc_sb = nrm.tile([64, CH], BF16, tag="rbc", name=f"rbc_sb{ch}_{t}_{hh}")
                    nc.vector.tensor_copy(rbc_sb[:], rbc_ps[:])
                    dst = OT[t][64 * hh: 64 * hh + 64, q0:q0 + CH]
                    nc.vector.tensor_mul(dst, ot_ps[hh][0:64, :], rbc_sb[:])
                    if has_bv:
                        nc.vector.tensor_scalar_add(dst, dst, bv_sb[64 * hh:64 * hh + 64, t:t + 1])
                # ship this pair's 128 OT rows for the quad exchange
                nc.sync.dma_start(ag_ins[ch][128 * t:128 * (t + 1), :], OT[t][:, q0:q0 + CH])
            nc.gpsimd.collective_compute(
                "AllGather",
                mybir.AluOpType.bypass,
                replica_groups=[[0, 1, 2, 3], [4, 5, 6, 7]],
                ins=[ag_ins[ch].opt()],
                outs=[ag_outs[ch].opt()],
            )

        def cproj_chunk(ch):
            g_sb = cpj.tile([128, NK * 128], BF16, tag="g", name=f"g{ch}")
            base = ag_outs[ch].rearrange("(k p) c -> p k c", k=NK)[:, :, 0:128]
            nc.sync.dma_start(
                g_sb.rearrange("p (k c) -> p k c", k=NK),
                AP(base.tensor, goff, base.ap, dep_tracking_offset=0),
            )
            for n in range(2):
                po = ps_mm.tile([128, CH], F32, tag="ps", name=f"po{ch}_{n}")
                for kt in range(NK):
                    nc.tensor.matmul(
                        po[:],
                        g_sb[:, kt * 128:(kt + 1) * 128],
                        wp_sb[:, kt * D + n * CH: kt * D + (n + 1) * CH],
                        start=(kt == 0), stop=(kt == NK - 1),
                    )
                ob = outp.tile([128, CH], F32, tag="ob", name=f"ob{ch}_{n}")
                if has_bp:
                    nc.vector.tensor_add(ob[:], po[:], bp_sb[:, n * CH:(n + 1) * CH])
                else:
                    nc.vector.tensor_copy(ob[:], po[:])
                nc.sync.dma_start(out_d[ch * 128:(ch + 1) * 128, n * CH:(n + 1) * CH], ob[:])

        for ch in range(NCH):
            qkv_chunk(ch)
            if ch > 0:
                cproj_chunk(ch - 1)
            attention_chunk(ch)
        cproj_chunk(NCH - 1)
        if dump_ot:
            for t in range(NRT):
                dbg_f32 = outp.tile([128, S], F32, tag="dbgf", name=f"dbgf{t}")
                nc.vector.tensor_copy(dbg_f32[:], OT[t][:])
                nc.sync.dma_start(dbg_d[:, t * S:(t + 1) * S], dbg_f32[:])

    nc.compile()
    return nc


_prog_cache = {}


def _get_prog(has_bv, has_bp, has_bqk):
    key = (has_bv, has_bp, has_bqk)
    if key not in _prog_cache:
        _prog_cache[key] = _build(has_bv, has_bp, has_bqk)
    return _prog_cache[key]


def _interleave(w, cols):
    # [D, cols] -> [128, NK*cols] with k-tile kt at column block kt
    return np.ascontiguousarray(
        w.reshape(NK, 128, cols).transpose(1, 0, 2).reshape(128, NK * cols)
    )


def _prepare(x, w_attn, b_attn, w_proj, b_proj):
    x = np.asarray(x, dtype=np.float32)
    w_attn = np.asarray(w_attn, dtype=np.float32)
    b_attn = np.asarray(b_attn, dtype=np.float32)
    w_proj = np.asarray(w_proj, dtype=np.float32)
    b_proj = np.asarray(b_proj, dtype=np.float32)

    has_bv = bool(np.any(b_attn[2 * D:]))
    has_bp = bool(np.any(b_proj))
    has_bqk = bool(np.any(b_attn[:2 * D]))
    nc = _get_prog(has_bv, has_bp, has_bqk)

    ii = np.arange(128)[:, None]
    jj = np.arange(128)[None, :]
    masks = np.zeros((128, 256), dtype=np.float32)
    masks[:, :128] = (jj >= ii)
    masks[0, 128:192] = 1.0   # ones row (base partition 0) for rden broadcast
    masks_bf = masks.astype(ml_dtypes.bfloat16)

    wp_il = _interleave(w_proj, D).astype(ml_dtypes.bfloat16)

    in_maps = []
    for c in range(N_CORES):
        b, g = divmod(c, 4)
        q0 = g * DG
        k0 = D + g * DG
        v0 = 2 * D + g * DG
        wv_ext = np.zeros((D, VW), dtype=np.float32)
        for hl in range(HG):
            wv_ext[:, hl * (HD + 1):hl * (HD + 1) + HD] = w_attn[:, v0 + hl * HD: v0 + (hl + 1) * HD]
        xt = x[b].T  # [D, S]
        xt_il = np.ascontiguousarray(
            xt.reshape(NK, 128, NCH, CH).transpose(1, 2, 0, 3).reshape(128, NCH * NK * CH)
        ).astype(ml_dtypes.bfloat16)
        in_maps.append({
            "xt": xt_il,
            "wq": _interleave(w_attn[:, q0:q0 + DG], DG).astype(ml_dtypes.bfloat16),
            "wk": _interleave(w_attn[:, k0:k0 + DG], DG).astype(ml_dtypes.bfloat16),
            "wv": _interleave(wv_ext, VW).astype(ml_dtypes.bfloat16),
            "wp": wp_il,
            "bq": np.ascontiguousarray(b_attn[q0:q0 + DG].reshape(NRT, 128).T),
            "bk": np.ascontiguousarray(b_attn[k0:k0 + DG].reshape(NRT, 128).T),
            "bv": np.ascontiguousarray(b_attn[v0:v0 + DG].reshape(NRT, 128).T),
            "bp": np.broadcast_to(b_proj, (128, D)).astype(np.float32).copy(),
            "masks": masks_bf,
        })
    return nc, in_maps


def _assemble(results):
    out = np.empty((B, S, D), dtype=np.float32)
    for c in range(N_CORES):
        b, g = divmod(c, 4)
        o = results[c]["out"]
        for ch in range(NCH):
            tok = ch * CH + g * 128
            out[b, tok:tok + 128, :] = o[ch * 128:(ch + 1) * 128, :]
    return out


def kernel(x, w_attn, b_attn, w_proj, b_proj):
    nc, in_maps = _prepare(x, w_attn, b_attn, w_proj, b_proj)
    res = run_bass_kernel_spmd(nc, in_maps, list(range(N_CORES)))
    return _assemble(res.results)


# revision 25
# speedup vs baseline: 1.5135x; 1.0305x over previous
"""GPT2 eager causal attention (B=2, S=2048, D=1024, H=16, HD=64) on 8 TRN2 NeuronCores.

Sharding (data + head/tensor parallel): core c -> (batch b = c//4, head-group
g = c%4) -- 4 heads per group; each quad (same batch) exchanges attention
outputs with a small bf16 AllToAll and computes the full c_proj locally for
its own token slices (no ReduceScatter, no fp32 partials).

Per-core pipeline:
  0. host pre-transposes x -> xT[d, s] and pre-interleaves every weight into
     its SBUF layout, so all device DMAs are large and contiguous.
  1. per 512-token chunk ch: QT/KT row-tiles and V strips via PE matmuls
     (d-contraction over 8 k-tiles), interleaved with...
  2. attention for chunk ch, head-pair t: score tiles for both heads of the
     pair go into one [128,1024] 2-bank PSUM tile via two row-group-packed
     K=64 matmuls (rows 0:64 / 64:128 -> concurrent on the PE sub-arrays);
     ONE exp covers both heads; diagonal k-tiles exp only the causal suffix
     and the st/ot matmuls skip the masked prefix columns entirely.
     V carries a ones-column per head so ot row 64 is the softmax denominator;
     reciprocal straight from PSUM, broadcast to 128 partitions with a K=2
     sel-matmul, normalize on DVE.
  3. OT chunk [256, 512]bf16 -> DRAM -> AllToAll over the quad (each core
     receives the full-model OT columns for its 128-token slice) -> local
     c_proj with the full w_proj -> fp32 out rows.
"""
from contextlib import ExitStack

import ml_dtypes
import numpy as np

import concourse.bacc as bacc
import concourse.mybir as mybir
import concourse.tile as tile
from concourse.bass import AP
from concourse.bass_utils import run_bass_kernel_spmd

F32 = mybir.dt.float32
BF16 = mybir.dt.bfloat16

B, S, D, H, HD = 2, 2048, 1024, 16, 64
N_CORES = 8
HG = 4               # heads per group (per core)
DG = HG * HD         # 256 q/k channels per group
VW = HG * (HD + 1)   # 260: 64 v-cols + 1 ones-col per head
NK = D // 128        # 8 contraction tiles over d
CH = 512             # q-chunk (one PSUM bank of fp32)
NCH = S // CH        # 4
NRT = DG // 128      # 2 channel row-tiles per group
WARMUP_MM = 64       # dummy matmuls to lift the PE HAM throttle before real work


def _build(has_bv: bool, has_bp: bool, has_bqk: bool = False, dump_ot: bool = False):
    nc = bacc.Bacc("TRN2", target_bir_lowering=False, debug=False, num_devices=N_CORES)
    dbg_d = nc.dram_tensor("dbg", [128, NRT * S], F32, kind="ExternalOutput").ap() if dump_ot else None

    xt_d = nc.dram_tensor("xt", [128, NCH * NK * CH], BF16, kind="ExternalInput").ap()
    wq_d = nc.dram_tensor("wq", [128, NK * DG], BF16, kind="ExternalInput").ap()
    wk_d = nc.dram_tensor("wk", [128, NK * DG], BF16, kind="ExternalInput").ap()
    wv_d = nc.dram_tensor("wv", [128, NK * VW], BF16, kind="ExternalInput").ap()
    wp_d = nc.dram_tensor("wp", [128, NK * D], BF16, kind="ExternalInput").ap()
    bq_d = nc.dram_tensor("bq", [128, NRT], F32, kind="ExternalInput").ap()
    bk_d = nc.dram_tensor("bk", [128, NRT], F32, kind="ExternalInput").ap()
    bv_d = nc.dram_tensor("bv", [128, NRT], F32, kind="ExternalInput").ap()
    bp_d = nc.dram_tensor("bp", [128, D], F32, kind="ExternalInput").ap()
    mk_d = nc.dram_tensor("masks", [128, 256], BF16, kind="ExternalInput").ap()
    out_d = nc.dram_tensor("out", [CH, D], F32, kind="ExternalOutput").ap()

    EXP = mybir.ActivationFunctionType.Exp
    IDENT = mybir.ActivationFunctionType.Identity

    with ExitStack() as ctx:
        tc = ctx.enter_context(tile.TileContext(nc))
        wpool = ctx.enter_context(tc.tile_pool(name="w", bufs=1))
        big = ctx.enter_context(tc.tile_pool(name="big", bufs=1))
        qkvp = ctx.enter_context(tc.tile_pool(name="qkv", bufs=1))
        stp = ctx.enter_context(tc.tile_pool(name="stx", bufs=4))
        nrm = ctx.enter_context(tc.tile_pool(name="nrm", bufs=2))
        cpj = ctx.enter_context(tc.tile_pool(name="cpj", bufs=2))
        outp = ctx.enter_context(tc.tile_pool(name="outp", bufs=3))
        ps_mm = ctx.enter_context(tc.tile_pool(name="psmm", bufs=2, space="PSUM"))
        ps_st = ctx.enter_context(tc.tile_pool(name="psst", bufs=2, space="PSUM"))
        ps_ot = ctx.enter_context(tc.tile_pool(name="psot", bufs=1, space="PSUM"))
        dram = ctx.enter_context(tc.tile_pool(name="dram", bufs=1, space="DRAM"))

        # ---- tiny constants first so the PE warm-up can start immediately
        # mk_sb cols 0:128 = triangular causal mask; rows 0:2 cols 128:256 =
        # the sel pattern that broadcasts rden row 0 -> partitions 0:64 and
        # row 1 -> partitions 64:128 via a K=2 matmul.
        mk_sb = wpool.tile([128, 256], BF16)
        nc.sync.dma_start(mk_sb[:], mk_d[:])
        if WARMUP_MM:
            warm_ps = ps_mm.tile([128, 512], F32, tag="ps", name="warm_ps")
            for i in range(WARMUP_MM):
                nc.tensor.matmul(
                    warm_ps[:, 0:256], mk_sb[:, 0:128], mk_sb[:, 0:256],
                    start=True, stop=True,
                )

        # ---- weights / x strips (host already laid out in SBUF order)
        xt_sb = big.tile([128, NCH * NK * CH], BF16, name="xt_sb")
        xt3 = xt_sb.rearrange("p (k c) -> p k c", k=NK)
        wq_sb = wpool.tile([128, NK * DG], BF16)
        wk_sb = wpool.tile([128, NK * DG], BF16)
        wv_sb = wpool.tile([128, NK * VW], BF16)
        wp_sb = wpool.tile([128, NK * D], BF16)
        bq_sb = wpool.tile([128, NRT], F32) if has_bqk else None
        bk_sb = wpool.tile([128, NRT], F32) if has_bqk else None
        bv_sb = wpool.tile([128, NRT], F32) if has_bv else None
        bp_sb = wpool.tile([128, D], F32) if has_bp else None

        def load_x_chunk(ch):
            # 4 sub-DMAs so the chunk spreads across parallel DMA queues
            for q in range(4):
                nc.sync.dma_start(
                    xt3[:, 2 * q:2 * q + 2, ch * CH:(ch + 1) * CH],
                    xt_d[:, ch * NK * CH + 2 * q * CH: ch * NK * CH + (2 * q + 2) * CH],
                )

        load_x_chunk(0)
        nc.sync.dma_start(wq_sb[:], wq_d[:])
        nc.sync.dma_start(wk_sb[:], wk_d[:])
        nc.sync.dma_start(wv_sb[:], wv_d[:])
        for ch in range(1, NCH):
            load_x_chunk(ch)
        nc.sync.dma_start(wp_sb[:, :NK * D // 2], wp_d[:, :NK * D // 2])
        nc.sync.dma_start(wp_sb[:, NK * D // 2:], wp_d[:, NK * D // 2:])
        if has_bqk:
            nc.sync.dma_start(bq_sb[:], bq_d[:])
            nc.sync.dma_start(bk_sb[:], bk_d[:])
        if has_bv:
            nc.sync.dma_start(bv_sb[:], bv_d[:])
        if has_bp:
            nc.sync.dma_start(bp_sb[:], bp_d[:])

        # ---- persistent SBUF tensors
        QT = [qkvp.tile([128, S], BF16, name=f"qT{rt}") for rt in range(NRT)]
        KT = [qkvp.tile([128, S], BF16, name=f"kT{rt}") for rt in range(NRT)]
        V = [qkvp.tile([128, VW], BF16, tag=f"v{st}", name=f"v{st}") for st in range(S // 128)]
        OT = [big.tile([128, S], BF16, name=f"OT{t}") for t in range(NRT)]

        def qkv_chunk(ch):
            # QT/KT row-tiles for this chunk's 512 tokens
            for store, w_sb, b_sb, nm in ((KT, wk_sb, bk_sb, "k"), (QT, wq_sb, bq_sb, "q")):
                for rt in range(NRT):
                    ps = ps_mm.tile([128, CH], F32, tag="ps", name=f"ps{nm}{rt}_{ch}")
                    for kt in range(NK):
                        nc.tensor.matmul(
                            ps[:],
                            w_sb[:, kt * DG + rt * 128: kt * DG + (rt + 1) * 128],
                            xt3[:, kt, ch * CH:(ch + 1) * CH],
                            start=(kt == 0), stop=(kt == NK - 1),
                        )
                    dst = store[rt][:, ch * CH:(ch + 1) * CH]
                    if has_bqk:
                        nc.scalar.activation(dst, ps[:], IDENT, bias=b_sb[:, rt:rt + 1])
                    else:
                        nc.vector.tensor_copy(dst, ps[:])
            # V strips
            for st in range(4 * ch, 4 * ch + 4):
                ps = ps_mm.tile([128, CH], F32, tag="ps", name=f"psv{st}")
                for kt in range(NK):
                    nc.tensor.matmul(
                        ps[:, :VW],
                        xt3[:, kt, st * 128:(st + 1) * 128],
                        wv_sb[:, kt * VW:(kt + 1) * VW],
                        start=(kt == 0), stop=(kt == NK - 1),
                    )
                vt = V[st]
                nc.vector.tensor_copy(vt[:], ps[:, :VW])
                for hl in range(HG):
                    ones_col = vt[:, hl * (HD + 1) + HD: (hl + 1) * (HD + 1)].bitcast(mybir.dt.uint16)
                    nc.vector.memset(ones_col, 0x3F80)  # bf16 1.0

        # DRAM staging for the per-chunk AllGather: each core contributes its
        # [256, 512] bf16 OT chunk and receives the full-model [1024, 512];
        # it then loads only the 128-token column slice it owns (dynamic
        # offset from the device id).
        ag_ins, ag_outs = [], []
        for ch in range(NCH):
            ai = [dram.tile([128, CH], BF16, tag=f"agi{ch}_{t}", name=f"ag_in{ch}_{t}") for t in range(NRT)]
            ao = [dram.tile([HG * 128, CH], BF16, tag=f"ago{ch}_{t}", name=f"ag_out{ch}_{t}") for t in range(NRT)]
            ag_ins.append(ai)
            ag_outs.append(ao)
        pid = nc.sync.partition_id()
        goff = (pid % HG) * 128  # my token-block column offset in the gathered chunk

        def att_pair(ch, t):
            """Attention for chunk ch, head pair t (heads 2t, 2t+1)."""
            q0 = ch * CH
            nkt = 4 * (ch + 1)
            ot_ps = {}
            ot_ps[0] = ps_ot.tile([HD + 1, CH], F32, tag="otA", name=f"otA{ch}_{t}")
            ot_ps[1] = ps_ot.tile([HD + 1, CH], F32, tag="otB", name=f"otB{ch}_{t}")
            for kt in range(nkt):
                d = kt - 4 * ch
                d0 = max(d, 0) * 128
                st_ps = ps_st.tile([128, 2 * CH], F32, tag="st", name=f"st{ch}_{t}_{kt}")
                st_sb = stp.tile([128, 2 * CH], BF16, tag="stsb", name=f"se{ch}_{t}_{kt}")
                for hh in range(2):  # row-group packed pair: concurrent on PE
                    off = 64 * hh
                    nc.tensor.matmul(
                        st_ps[:, hh * CH + d0: (hh + 1) * CH],
                        KT[t][off:off + 64, kt * 128:(kt + 1) * 128],
                        QT[t][off:off + 64, q0 + d0: q0 + CH],
                        start=True, stop=True,
                    )
                if d <= 0:
                    # one exp covers both heads' 512-column halves
                    nc.scalar.activation(st_sb[:], st_ps[:], EXP, scale=0.125)
                else:
                    for hh in range(2):
                        nc.scalar.activation(
                            st_sb[:, hh * CH + d0: (hh + 1) * CH],
                            st_ps[:, hh * CH + d0: (hh + 1) * CH],
                            EXP, scale=0.125,
                        )
                if d >= 0:
                    for hh in range(2):
                        blk = st_sb[:, hh * CH + d0: hh * CH + d0 + 128]
                        nc.vector.tensor_mul(blk, blk, mk_sb[:, 0:128])
                for hh in range(2):
                    hl = 2 * t + hh
                    nc.tensor.matmul(
                        ot_ps[hh][:, d0:],
                        V[kt][:, hl * (HD + 1):(hl + 1) * (HD + 1)],
                        st_sb[:, hh * CH + d0:(hh + 1) * CH],
                        start=(kt == 0), stop=(kt == nkt - 1),
                    )
            # normalize: row 64 of each ot_ps is the softmax denominator.
            # reciprocal_approx_fast ignores PSUM partition offsets, so bounce
            # the denominator rows through SBUF; the ot banks are released as
            # soon as the den + ot_sb copies drain (muls then read SBUF only).
            den = nrm.tile([1, 2 * CH], F32, tag="den", name=f"den{ch}_{t}")
            rden = nrm.tile([1, 2 * CH], F32, tag="rden", name=f"rden{ch}_{t}")
            rden_bf = nrm.tile([1, 2 * CH], BF16, tag="rdenb", name=f"rdenb{ch}_{t}")
            ot_sb = nrm.tile([128, CH], BF16, tag="otsb", name=f"ot_sb{ch}_{t}")
            nc.vector.tensor_copy(den[0:1, 0:CH], ot_ps[0][64:65, :])
            nc.vector.tensor_copy(den[0:1, CH:2 * CH], ot_ps[1][64:65, :])
            for hh in range(2):
                nc.vector.tensor_copy(ot_sb[64 * hh:64 * hh + 64, :], ot_ps[hh][0:64, :])
            nc.vector.reciprocal_approx_fast(rden[:], den[:])
            nc.vector.tensor_copy(rden_bf[:], rden[:])
            for hh in range(2):
                rbc_ps = ps_mm.tile([64, CH], F32, tag="ps", name=f"rbc{ch}_{t}_{hh}")
                nc.tensor.matmul(
                    rbc_ps[:],
                    mk_sb[0:1, 128:192],
                    rden_bf[0:1, hh * CH:(hh + 1) * CH],
                    start=True, stop=True,
                )
                rbc_sb = nrm.tile([128, CH], BF16, tag="rbc", name=f"rbc_sb{ch}_{t}_{hh}", bufs=2)
                nc.vector.tensor_copy(rbc_sb[64 * hh:64 * hh + 64, :], rbc_ps[:])
                dst = OT[t][64 * hh: 64 * hh + 64, q0:q0 + CH]
                nc.vector.tensor_mul(dst, ot_sb[64 * hh:64 * hh + 64, :], rbc_sb[64 * hh:64 * hh + 64, :])
                if has_bv:
                    nc.vector.tensor_scalar_add(dst, dst, bv_sb[64 * hh:64 * hh + 64, t:t + 1])
            # ship this pair's 128 OT rows; each pair has its own AllGather so
            # the exchange streams while the other pair still computes
            nc.sync.dma_start(ag_ins[ch][t][:], OT[t][:, q0:q0 + CH])
            nc.gpsimd.collective_compute(
                "AllGather",
                mybir.AluOpType.bypass,
                replica_groups=[[0, 1, 2, 3], [4, 5, 6, 7]],
                ins=[ag_ins[ch][t].opt()],
                outs=[ag_outs[ch][t].opt()],
            )

        def cproj_chunk(ch):
            # gathered pair-t AG covers my token block's OT rows for k-tiles
            # {2r + t}; two dynamic-offset loads, then 2x8 accumulating matmuls
            g_sb = [cpj.tile([128, HG * 128], BF16, tag=f"g{t}", name=f"g{ch}_{t}") for t in range(NRT)]
            for t in range(NRT):
                base = ag_outs[ch][t].rearrange("(r p) c -> p r c", r=HG)[:, :, 0:128]
                nc.sync.dma_start(
                    g_sb[t].rearrange("p (r c) -> p r c", r=HG),
                    AP(base.tensor, goff, base.ap, dep_tracking_offset=0),
                )
            kt_order = [kt for kt in range(NK) if kt % 2 == 0] + [kt for kt in range(NK) if kt % 2 == 1]
            for n in range(2):
                po = ps_mm.tile([128, CH], F32, tag="ps", name=f"po{ch}_{n}")
                for i, kt in enumerate(kt_order):  # even k-tiles first: they only need pair-0's AG
                    t, r = kt % 2, kt // 2
                    nc.tensor.matmul(
                        po[:],
                        g_sb[t][:, r * 128:(r + 1) * 128],
                        wp_sb[:, kt * D + n * CH: kt * D + (n + 1) * CH],
                        start=(i == 0), stop=(i == NK - 1),
                    )
                ob = outp.tile([128, CH], F32, tag="ob", name=f"ob{ch}_{n}")
                if has_bp:
                    nc.vector.tensor_add(ob[:], po[:], bp_sb[:, n * CH:(n + 1) * CH])
                else:
                    nc.vector.tensor_copy(ob[:], po[:])
                nc.sync.dma_start(out_d[ch * 128:(ch + 1) * 128, n * CH:(n + 1) * CH], ob[:])

        for ch in range(NCH):
            qkv_chunk(ch)
            att_pair(ch, 0)
            if ch > 0:
                cproj_chunk(ch - 1)
            att_pair(ch, 1)
        cproj_chunk(NCH - 1)
        if dump_ot:
            for t in range(NRT):
                dbg_f32 = outp.tile([128, S], F32, tag="dbgf", name=f"dbgf{t}")
                nc.vector.tensor_copy(dbg_f32[:], OT[t][:])
                nc.sync.dma_start(dbg_d[:, t * S:(t + 1) * S], dbg_f32[:])

    nc.compile()
    return nc


_prog_cache = {}


def _get_prog(has_bv, has_bp, has_bqk):
    key = (has_bv, has_bp, has_bqk)
    if key not in _prog_cache:
        _prog_cache[key] = _build(has_bv, has_bp, has_bqk)
    return _prog_cache[key]


def _interleave(w, cols):
    # [D, cols] -> [128, NK*cols] with k-tile kt at column block kt
    return np.ascontiguousarray(
        w.reshape(NK, 128, cols).transpose(1, 0, 2).reshape(128, NK * cols)
    )


def _prepare(x, w_attn, b_attn, w_proj, b_proj):
    x = np.asarray(x, dtype=np.float32)
    w_attn = np.asarray(w_attn, dtype=np.float32)
    b_attn = np.asarray(b_attn, dtype=np.float32)
    w_proj = np.asarray(w_proj, dtype=np.float32)
    b_proj = np.asarray(b_proj, dtype=np.float32)

    has_bv = bool(np.any(b_attn[2 * D:]))
    has_bp = bool(np.any(b_proj))
    has_bqk = bool(np.any(b_attn[:2 * D]))
    nc = _get_prog(has_bv, has_bp, has_bqk)

    ii = np.arange(128)[:, None]
    jj = np.arange(128)[None, :]
    masks = np.zeros((128, 256), dtype=np.float32)
    masks[:, :128] = (jj >= ii)
    masks[0, 128:192] = 1.0   # ones row (base partition 0) for rden broadcast
    masks_bf = masks.astype(ml_dtypes.bfloat16)

    wp_il = _interleave(w_proj, D).astype(ml_dtypes.bfloat16)

    in_maps = []
    for c in range(N_CORES):
        b, g = divmod(c, 4)
        q0 = g * DG
        k0 = D + g * DG
        v0 = 2 * D + g * DG
        wv_ext = np.zeros((D, VW), dtype=np.float32)
        for hl in range(HG):
            wv_ext[:, hl * (HD + 1):hl * (HD + 1) + HD] = w_attn[:, v0 + hl * HD: v0 + (hl + 1) * HD]
        xt = x[b].T  # [D, S]
        xt_il = np.ascontiguousarray(
            xt.reshape(NK, 128, NCH, CH).transpose(1, 2, 0, 3).reshape(128, NCH * NK * CH)
        ).astype(ml_dtypes.bfloat16)
        in_maps.append({
            "xt": xt_il,
            "wq": _interleave(w_attn[:, q0:q0 + DG], DG).astype(ml_dtypes.bfloat16),
            "wk": _interleave(w_attn[:, k0:k0 + DG], DG).astype(ml_dtypes.bfloat16),
            "wv": _interleave(wv_ext, VW).astype(ml_dtypes.bfloat16),
            "wp": wp_il,
            "bq": np.ascontiguousarray(b_attn[q0:q0 + DG].reshape(NRT, 128).T),
            "bk": np.ascontiguousarray(b_attn[k0:k0 + DG].reshape(NRT, 128).T),
            "bv": np.ascontiguousarray(b_attn[v0:v0 + DG].reshape(NRT, 128).T),
            "bp": np.broadcast_to(b_proj, (128, D)).astype(np.float32).copy(),
            "masks": masks_bf,
        })
    return nc, in_maps


def _assemble(results):
    out = np.empty((B, S, D), dtype=np.float32)
    for c in range(N_CORES):
        b, g = divmod(c, 4)
        o = results[c]["out"]
        for ch in range(NCH):
            tok = ch * CH + g * 128
            out[b, tok:tok + 128, :] = o[ch * 128:(ch + 1) * 128, :]
    return out


def kernel(x, w_attn, b_attn, w_proj, b_proj):
    nc, in_maps = _prepare(x, w_attn, b_attn, w_proj, b_proj)
    res = run_bass_kernel_spmd(nc, in_maps, list(range(N_CORES)))
    return _assemble(res.results)


# revision 26
# speedup vs baseline: 1.5227x; 1.0060x over previous
"""GPT2 eager causal attention (B=2, S=2048, D=1024, H=16, HD=64) on 8 TRN2 NeuronCores.

Sharding (data + head/tensor parallel): core c -> (batch b = c//4, head-group
g = c%4) -- 4 heads per group; each quad (same batch) exchanges attention
outputs with a small bf16 AllToAll and computes the full c_proj locally for
its own token slices (no ReduceScatter, no fp32 partials).

Per-core pipeline:
  0. host pre-transposes x -> xT[d, s] and pre-interleaves every weight into
     its SBUF layout, so all device DMAs are large and contiguous.
  1. per 512-token chunk ch: QT/KT row-tiles and V strips via PE matmuls
     (d-contraction over 8 k-tiles), interleaved with...
  2. attention for chunk ch, head-pair t: score tiles for both heads of the
     pair go into one [128,1024] 2-bank PSUM tile via two row-group-packed
     K=64 matmuls (rows 0:64 / 64:128 -> concurrent on the PE sub-arrays);
     ONE exp covers both heads; diagonal k-tiles exp only the causal suffix
     and the st/ot matmuls skip the masked prefix columns entirely.
     V carries a ones-column per head so ot row 64 is the softmax denominator;
     reciprocal straight from PSUM, broadcast to 128 partitions with a K=2
     sel-matmul, normalize on DVE.
  3. OT chunk [256, 512]bf16 -> DRAM -> AllToAll over the quad (each core
     receives the full-model OT columns for its 128-token slice) -> local
     c_proj with the full w_proj -> fp32 out rows.
"""
from contextlib import ExitStack

import ml_dtypes
import numpy as np

import concourse.bacc as bacc
import concourse.mybir as mybir
import concourse.tile as tile
from concourse.bass import AP
from concourse.bass_utils import run_bass_kernel_spmd

F32 = mybir.dt.float32
BF16 = mybir.dt.bfloat16

B, S, D, H, HD = 2, 2048, 1024, 16, 64
N_CORES = 8
HG = 4               # heads per group (per core)
DG = HG * HD         # 256 q/k channels per group
VW = HG * (HD + 1)   # 260: 64 v-cols + 1 ones-col per head
NK = D // 128        # 8 contraction tiles over d
CH = 512             # q-chunk (one PSUM bank of fp32)
NCH = S // CH        # 4
NRT = DG // 128      # 2 channel row-tiles per group
WARMUP_MM = 40       # dummy matmuls to lift the PE HAM throttle before real work


def _build(has_bv: bool, has_bp: bool, has_bqk: bool = False, dump_ot: bool = False):
    nc = bacc.Bacc("TRN2", target_bir_lowering=False, debug=False, num_devices=N_CORES)
    dbg_d = nc.dram_tensor("dbg", [128, NRT * S], F32, kind="ExternalOutput").ap() if dump_ot else None

    xt_d = nc.dram_tensor("xt", [128, NCH * NK * CH], BF16, kind="ExternalInput").ap()
    wq_d = nc.dram_tensor("wq", [128, NK * DG], BF16, kind="ExternalInput").ap()
    wk_d = nc.dram_tensor("wk", [128, NK * DG], BF16, kind="ExternalInput").ap()
    wv_d = nc.dram_tensor("wv", [128, NK * VW], BF16, kind="ExternalInput").ap()
    wp_d = nc.dram_tensor("wp", [128, NK * D], BF16, kind="ExternalInput").ap()
    bq_d = nc.dram_tensor("bq", [128, NRT], F32, kind="ExternalInput").ap()
    bk_d = nc.dram_tensor("bk", [128, NRT], F32, kind="ExternalInput").ap()
    bv_d = nc.dram_tensor("bv", [128, NRT], F32, kind="ExternalInput").ap()
    bp_d = nc.dram_tensor("bp", [128, D], F32, kind="ExternalInput").ap()
    mk_d = nc.dram_tensor("masks", [128, 512], BF16, kind="ExternalInput").ap()
    out_d = nc.dram_tensor("out", [CH, D], F32, kind="ExternalOutput").ap()

    EXP = mybir.ActivationFunctionType.Exp
    IDENT = mybir.ActivationFunctionType.Identity

    with ExitStack() as ctx:
        tc = ctx.enter_context(tile.TileContext(nc))
        wpool = ctx.enter_context(tc.tile_pool(name="w", bufs=1))
        big = ctx.enter_context(tc.tile_pool(name="big", bufs=1))
        qkvp = ctx.enter_context(tc.tile_pool(name="qkv", bufs=1))
        stp = ctx.enter_context(tc.tile_pool(name="stx", bufs=4))
        nrm = ctx.enter_context(tc.tile_pool(name="nrm", bufs=2))
        cpj = ctx.enter_context(tc.tile_pool(name="cpj", bufs=2))
        outp = ctx.enter_context(tc.tile_pool(name="outp", bufs=3))
        ps_mm = ctx.enter_context(tc.tile_pool(name="psmm", bufs=2, space="PSUM"))
        ps_st = ctx.enter_context(tc.tile_pool(name="psst", bufs=2, space="PSUM"))
        ps_ot = ctx.enter_context(tc.tile_pool(name="psot", bufs=1, space="PSUM"))
        dram = ctx.enter_context(tc.tile_pool(name="dram", bufs=1, space="DRAM"))

        # ---- tiny constants first so the PE warm-up can start immediately
        # mk_sb cols 0:128 = triangular causal mask; rows 0:2 cols 128:256 =
        # the sel pattern that broadcasts rden row 0 -> partitions 0:64 and
        # row 1 -> partitions 64:128 via a K=2 matmul.
        mk_sb = wpool.tile([128, 512], BF16)
        nc.sync.dma_start(mk_sb[:], mk_d[:])
        if WARMUP_MM:
            warm_ps = ps_mm.tile([128, 512], F32, tag="ps", name="warm_ps")
            for i in range(WARMUP_MM):
                nc.tensor.matmul(
                    warm_ps[:, 0:256], mk_sb[:, 256:384], mk_sb[:, 0:256],
                    start=True, stop=True,
                )

        # ---- weights / x strips (host already laid out in SBUF order)
        xt_sb = big.tile([128, NCH * NK * CH], BF16, name="xt_sb")
        xt3 = xt_sb.rearrange("p (k c) -> p k c", k=NK)
        wq_sb = wpool.tile([128, NK * DG], BF16)
        wk_sb = wpool.tile([128, NK * DG], BF16)
        wv_sb = wpool.tile([128, NK * VW], BF16)
        wp_sb = wpool.tile([128, NK * D], BF16)
        bq_sb = wpool.tile([128, NRT], F32) if has_bqk else None
        bk_sb = wpool.tile([128, NRT], F32) if has_bqk else None
        bv_sb = wpool.tile([128, NRT], F32) if has_bv else None
        bp_sb = wpool.tile([128, D], F32) if has_bp else None

        def load_x_chunk(ch, eng):
            # 4 sub-DMAs so the chunk spreads across parallel DMA queues
            for q in range(4):
                eng.dma_start(
                    xt3[:, 2 * q:2 * q + 2, ch * CH:(ch + 1) * CH],
                    xt_d[:, ch * NK * CH + 2 * q * CH: ch * NK * CH + (2 * q + 2) * CH],
                )

        # sync HWDGE carries chunk 0 + qkv weights; the scalar HWDGE queue
        # (idle until the first exp) carries the rest -> 2x prologue bandwidth
        load_x_chunk(0, nc.sync)
        nc.sync.dma_start(wq_sb[:], wq_d[:])
        nc.sync.dma_start(wk_sb[:], wk_d[:])
        nc.sync.dma_start(wv_sb[:], wv_d[:])
        for ch in range(1, NCH):
            load_x_chunk(ch, nc.scalar)
        nc.scalar.dma_start(wp_sb[:, :NK * D // 2], wp_d[:, :NK * D // 2])
        nc.scalar.dma_start(wp_sb[:, NK * D // 2:], wp_d[:, NK * D // 2:])
        if has_bqk:
            nc.sync.dma_start(bq_sb[:], bq_d[:])
            nc.sync.dma_start(bk_sb[:], bk_d[:])
        if has_bv:
            nc.sync.dma_start(bv_sb[:], bv_d[:])
        if has_bp:
            nc.sync.dma_start(bp_sb[:], bp_d[:])

        # ---- persistent SBUF tensors
        QT = [qkvp.tile([128, S], BF16, name=f"qT{rt}") for rt in range(NRT)]
        KT = [qkvp.tile([128, S], BF16, name=f"kT{rt}") for rt in range(NRT)]
        V = [qkvp.tile([128, VW], BF16, tag=f"v{st}", name=f"v{st}") for st in range(S // 128)]
        OT = [big.tile([128, S], BF16, name=f"OT{t}") for t in range(NRT)]

        def qkv_chunk(ch):
            # QT/KT row-tiles for this chunk's 512 tokens
            for store, w_sb, b_sb, nm in ((KT, wk_sb, bk_sb, "k"), (QT, wq_sb, bq_sb, "q")):
                for rt in range(NRT):
                    ps = ps_mm.tile([128, CH], F32, tag="ps", name=f"ps{nm}{rt}_{ch}")
                    for kt in range(NK):
                        nc.tensor.matmul(
                            ps[:],
                            w_sb[:, kt * DG + rt * 128: kt * DG + (rt + 1) * 128],
                            xt3[:, kt, ch * CH:(ch + 1) * CH],
                            start=(kt == 0), stop=(kt == NK - 1),
                        )
                    dst = store[rt][:, ch * CH:(ch + 1) * CH]
                    if has_bqk:
                        nc.scalar.activation(dst, ps[:], IDENT, bias=b_sb[:, rt:rt + 1])
                    else:
                        # scalar engine: it idles during qkv production and
                        # this keeps the DVE free for the normalize chain
                        nc.scalar.copy(dst, ps[:])
            # V strips
            for st in range(4 * ch, 4 * ch + 4):
                ps = ps_mm.tile([128, CH], F32, tag="ps", name=f"psv{st}")
                for kt in range(NK):
                    nc.tensor.matmul(
                        ps[:, :VW],
                        xt3[:, kt, st * 128:(st + 1) * 128],
                        wv_sb[:, kt * VW:(kt + 1) * VW],
                        start=(kt == 0), stop=(kt == NK - 1),
                    )
                vt = V[st]
                nc.vector.tensor_copy(vt[:], ps[:, :VW])
                for hl in range(HG):
                    ones_col = vt[:, hl * (HD + 1) + HD: (hl + 1) * (HD + 1)].bitcast(mybir.dt.uint16)
                    nc.vector.memset(ones_col, 0x3F80)  # bf16 1.0

        # DRAM staging for the per-chunk AllGather: each core contributes its
        # [256, 512] bf16 OT chunk and receives the full-model [1024, 512];
        # it then loads only the 128-token column slice it owns (dynamic
        # offset from the device id).
        ag_ins, ag_outs = [], []
        for ch in range(NCH):
            ai = dram.tile([DG, CH], BF16, tag=f"agi{ch}", name=f"ag_in{ch}")
            ao = dram.tile([HG * DG, CH], BF16, tag=f"ago{ch}", name=f"ag_out{ch}")
            ag_ins.append(ai)
            ag_outs.append(ao)
        pid = nc.sync.partition_id()
        goff = (pid % HG) * 128  # my token-block column offset in the gathered chunk

        def att_pair(ch, t):
            """Attention for chunk ch, head pair t (heads 2t, 2t+1)."""
            q0 = ch * CH
            nkt = 4 * (ch + 1)
            ot_ps = {}
            ot_ps[0] = ps_ot.tile([HD + 1, CH], F32, tag="otA", name=f"otA{ch}_{t}")
            ot_ps[1] = ps_ot.tile([HD + 1, CH], F32, tag="otB", name=f"otB{ch}_{t}")
            for kt in range(nkt):
                d = kt - 4 * ch
                d0 = max(d, 0) * 128
                st_ps = ps_st.tile([128, 2 * CH], F32, tag="st", name=f"st{ch}_{t}_{kt}")
                st_sb = stp.tile([128, 2 * CH], BF16, tag="stsb", name=f"se{ch}_{t}_{kt}")
                diag = d >= 0
                for hh in range(2):  # row-group packed pair: concurrent on PE
                    off = 64 * hh
                    nc.tensor.matmul(
                        st_ps[:, hh * CH + d0: (hh + 1) * CH],
                        KT[t][off:off + 64, kt * 128:(kt + 1) * 128],
                        QT[t][off:off + 64, q0 + d0: q0 + CH],
                        start=True, stop=not diag,
                    )
                if diag:
                    # causal mask: PE-accumulate -1e9 onto the diagonal block
                    # (identity @ maskneg), so exp gives exact zeros and the
                    # DVE stays out of the exp->ot chain
                    for hh in range(2):
                        nc.tensor.matmul(
                            st_ps[:, hh * CH + d0: hh * CH + d0 + 128],
                            mk_sb[:, 256:384],
                            mk_sb[:, 0:128],
                            start=False, stop=True,
                        )
                if d <= 0:
                    # one exp covers both heads' 512-column halves
                    nc.scalar.activation(st_sb[:], st_ps[:], EXP, scale=0.125)
                else:
                    for hh in range(2):
                        nc.scalar.activation(
                            st_sb[:, hh * CH + d0: (hh + 1) * CH],
                            st_ps[:, hh * CH + d0: (hh + 1) * CH],
                            EXP, scale=0.125,
                        )
                for hh in range(2):
                    hl = 2 * t + hh
                    nc.tensor.matmul(
                        ot_ps[hh][:, d0:],
                        V[kt][:, hl * (HD + 1):(hl + 1) * (HD + 1)],
                        st_sb[:, hh * CH + d0:(hh + 1) * CH],
                        start=(kt == 0), stop=(kt == nkt - 1),
                    )
            # normalize: row 64 of each ot_ps is the softmax denominator.
            # reciprocal_approx_fast ignores PSUM partition offsets, so bounce
            # the denominator rows through SBUF; the ot banks are released as
            # soon as the den + ot_sb copies drain (muls then read SBUF only).
            den = nrm.tile([1, 2 * CH], F32, tag="den", name=f"den{ch}_{t}")
            rden = nrm.tile([1, 2 * CH], F32, tag="rden", name=f"rden{ch}_{t}")
            rden_bf = nrm.tile([1, 2 * CH], BF16, tag="rdenb", name=f"rdenb{ch}_{t}")
            ot_sb = nrm.tile([128, CH], BF16, tag="otsb", name=f"ot_sb{ch}_{t}")
            nc.vector.tensor_copy(den[0:1, 0:CH], ot_ps[0][64:65, :])
            nc.vector.tensor_copy(den[0:1, CH:2 * CH], ot_ps[1][64:65, :])
            for hh in range(2):
                nc.vector.tensor_copy(ot_sb[64 * hh:64 * hh + 64, :], ot_ps[hh][0:64, :])
            nc.vector.reciprocal_approx_fast(rden[:], den[:])
            nc.vector.tensor_copy(rden_bf[:], rden[:])
            for hh in range(2):
                rbc_ps = ps_mm.tile([64, CH], F32, tag="ps", name=f"rbc{ch}_{t}_{hh}")
                nc.tensor.matmul(
                    rbc_ps[:],
                    mk_sb[0:1, 128:192],
                    rden_bf[0:1, hh * CH:(hh + 1) * CH],
                    start=True, stop=True,
                )
                rbc_sb = nrm.tile([128, CH], BF16, tag="rbc", name=f"rbc_sb{ch}_{t}_{hh}", bufs=2)
                nc.vector.tensor_copy(rbc_sb[64 * hh:64 * hh + 64, :], rbc_ps[:])
                dst = OT[t][64 * hh: 64 * hh + 64, q0:q0 + CH]
                nc.vector.tensor_mul(dst, ot_sb[64 * hh:64 * hh + 64, :], rbc_sb[64 * hh:64 * hh + 64, :])
                if has_bv:
                    nc.vector.tensor_scalar_add(dst, dst, bv_sb[64 * hh:64 * hh + 64, t:t + 1])
            # ship this pair's 128 OT rows for the quad exchange
            nc.sync.dma_start(ag_ins[ch][128 * t:128 * (t + 1), :], OT[t][:, q0:q0 + CH])
            if t == NRT - 1:
                nc.gpsimd.collective_compute(
                    "AllGather",
                    mybir.AluOpType.bypass,
                    replica_groups=[[0, 1, 2, 3], [4, 5, 6, 7]],
                    ins=[ag_ins[ch].opt()],
                    outs=[ag_outs[ch].opt()],
                )

        def cproj_chunk(ch):
            g_sb = cpj.tile([128, NK * 128], BF16, tag="g", name=f"g{ch}")
            base = ag_outs[ch].rearrange("(k p) c -> p k c", k=NK)[:, :, 0:128]
            nc.sync.dma_start(
                g_sb.rearrange("p (k c) -> p k c", k=NK),
                AP(base.tensor, goff, base.ap, dep_tracking_offset=0),
            )
            for n in range(2):
                po = ps_mm.tile([128, CH], F32, tag="ps", name=f"po{ch}_{n}")
                for kt in range(NK):
                    nc.tensor.matmul(
                        po[:],
                        g_sb[:, kt * 128:(kt + 1) * 128],
                        wp_sb[:, kt * D + n * CH: kt * D + (n + 1) * CH],
                        start=(kt == 0), stop=(kt == NK - 1),
                    )
                ob = outp.tile([128, CH], F32, tag="ob", name=f"ob{ch}_{n}")
                if has_bp:
                    nc.vector.tensor_add(ob[:], po[:], bp_sb[:, n * CH:(n + 1) * CH])
                else:
                    nc.vector.tensor_copy(ob[:], po[:])
                nc.sync.dma_start(out_d[ch * 128:(ch + 1) * 128, n * CH:(n + 1) * CH], ob[:])

        # attention order 1, 2, 3, 0: the final exposed AllGather+c_proj tail
        # belongs to the smallest chunk (0); each chunk's qkv is produced just
        # ahead of its attention, and c_proj lags one attention chunk behind
        qkv_chunk(0)
        qkv_chunk(1)
        att_pair(1, 0)
        qkv_chunk(2)
        att_pair(1, 1)
        att_pair(2, 0)
        qkv_chunk(3)
        att_pair(2, 1)
        cproj_chunk(1)
        att_pair(3, 0)
        att_pair(3, 1)
        cproj_chunk(2)
        att_pair(0, 0)
        att_pair(0, 1)
        cproj_chunk(3)
        cproj_chunk(0)
        if dump_ot:
            for t in range(NRT):
                dbg_f32 = outp.tile([128, S], F32, tag="dbgf", name=f"dbgf{t}")
                nc.vector.tensor_copy(dbg_f32[:], OT[t][:])
                nc.sync.dma_start(dbg_d[:, t * S:(t + 1) * S], dbg_f32[:])

    nc.compile()
    return nc


_prog_cache = {}


def _get_prog(has_bv, has_bp, has_bqk):
    key = (has_bv, has_bp, has_bqk)
    if key not in _prog_cache:
        _prog_cache[key] = _build(has_bv, has_bp, has_bqk)
    return _prog_cache[key]


def _interleave(w, cols):
    # [D, cols] -> [128, NK*cols] with k-tile kt at column block kt
    return np.ascontiguousarray(
        w.reshape(NK, 128, cols).transpose(1, 0, 2).reshape(128, NK * cols)
    )


def _prepare(x, w_attn, b_attn, w_proj, b_proj):
    x = np.asarray(x, dtype=np.float32)
    w_attn = np.asarray(w_attn, dtype=np.float32)
    b_attn = np.asarray(b_attn, dtype=np.float32)
    w_proj = np.asarray(w_proj, dtype=np.float32)
    b_proj = np.asarray(b_proj, dtype=np.float32)

    has_bv = bool(np.any(b_attn[2 * D:]))
    has_bp = bool(np.any(b_proj))
    has_bqk = bool(np.any(b_attn[:2 * D]))
    nc = _get_prog(has_bv, has_bp, has_bqk)

    ii = np.arange(128)[:, None]
    jj = np.arange(128)[None, :]
    masks = np.zeros((128, 512), dtype=np.float32)
    masks[:, :128] = np.where(jj >= ii, 0.0, -1e9)  # additive causal mask
    masks[0, 128:192] = 1.0   # ones row (base partition 0) for rden broadcast
    masks[:, 256:384] = (ii == jj)  # identity for the mask accumulate-matmul
    masks_bf = masks.astype(ml_dtypes.bfloat16)

    wp_il = _interleave(w_proj, D).astype(ml_dtypes.bfloat16)

    in_maps = []
    for c in range(N_CORES):
        b, g = divmod(c, 4)
        q0 = g * DG
        k0 = D + g * DG
        v0 = 2 * D + g * DG
        wv_ext = np.zeros((D, VW), dtype=np.float32)
        for hl in range(HG):
            wv_ext[:, hl * (HD + 1):hl * (HD + 1) + HD] = w_attn[:, v0 + hl * HD: v0 + (hl + 1) * HD]
        xt = x[b].T  # [D, S]
        xt_il = np.ascontiguousarray(
            xt.reshape(NK, 128, NCH, CH).transpose(1, 2, 0, 3).reshape(128, NCH * NK * CH)
        ).astype(ml_dtypes.bfloat16)
        in_maps.append({
            "xt": xt_il,
            "wq": _interleave(w_attn[:, q0:q0 + DG], DG).astype(ml_dtypes.bfloat16),
            "wk": _interleave(w_attn[:, k0:k0 + DG], DG).astype(ml_dtypes.bfloat16),
            "wv": _interleave(wv_ext, VW).astype(ml_dtypes.bfloat16),
            "wp": wp_il,
            "bq": np.ascontiguousarray(b_attn[q0:q0 + DG].reshape(NRT, 128).T),
            "bk": np.ascontiguousarray(b_attn[k0:k0 + DG].reshape(NRT, 128).T),
            "bv": np.ascontiguousarray(b_attn[v0:v0 + DG].reshape(NRT, 128).T),
            "bp": np.broadcast_to(b_proj, (128, D)).astype(np.float32).copy(),
            "masks": masks_bf,
        })
    return nc, in_maps


def _assemble(results):
    out = np.empty((B, S, D), dtype=np.float32)
    for c in range(N_CORES):
        b, g = divmod(c, 4)
        o = results[c]["out"]
        for ch in range(NCH):
            tok = ch * CH + g * 128
            out[b, tok:tok + 128, :] = o[ch * 128:(ch + 1) * 128, :]
    return out


def kernel(x, w_attn, b_attn, w_proj, b_proj):
    nc, in_maps = _prepare(x, w_attn, b_attn, w_proj, b_proj)
    res = run_bass_kernel_spmd(nc, in_maps, list(range(N_CORES)))
    return _assemble(res.results)
